# revision 1
# baseline (speedup 1.0000x reference)
"""Trainium2 Bass kernel for BilinearClassification (segment_reduce).

Math (per example b):
  ent[e,:]  = masked-mean over subword span of hidden[idx[e,s],:]      (E=64, H=768)
  subj[t,:] = ent[trip[t,0],:] * pm[t];  obj[t,:] = ent[trip[t,1],:] * pm[t]
  bl[t, (g,i,j)] = subj[t, g*8+i] * obj[t, g*8+j]                      (f = 6144)
  logits[t,n] = bl[t,:] @ W[:,n] + b[n]                                (NT=42)

Device strategy (8 cores, 4 examples = 2 example-pairs each, no collectives):
  - Only VALID triplets (pair_mask) are computed, packed into C<=128 columns
    per example-pair (host pairs examples to balance; the few overflow
    triplets are computed exactly on the host). Masked slots get b_fc.
  - Host gathers only the DISTINCT hidden rows each example references
    (K*128 rows instead of L=512), shrinking hid DMA and stage-1 PE.
  - stage 1: ent[(ex,e), h] = AT.T @ hidg per pair (2K contraction chunks).
  - stage 2 per pair, TRIPLET-MAJOR (t on partitions):
      subjT[t, h] = G_s.T @ ent   (2 matmuls), evac SBUF bf16; objT likewise.
      bl_t[t, (g,i,j)] = subjT[t,(g,i)] * objT[t,(g,j)]  -- broadcast-AP
          elementwise muls, split across DVE and GPSIMD (no PE, no PSUM).
      per 8-chunk group: PE-transpose bl_t chunks to [f, t] (PSUM bf16),
          evac (ACT/DVE alternating), logits += W_c.T @ blT_c.
  - host scatters packed columns back to (b, t) and adds b_fc.
"""
import sys

sys.path.insert(0, "/opt/trn_rl_repo")

import numpy as np

import concourse.bass as bass
import concourse.bacc as bacc
import concourse.tile as tile
from concourse import mybir
from concourse.bass_utils import run_bass_kernel_spmd

F32 = mybir.dt.float32
BF16 = mybir.dt.bfloat16

B, L, H = 32, 512, 768
E, S, T = 64, 8, 128
NT = 42
NCORES = 8
EXPC = B // NCORES          # 4 examples per core
NPAIR = EXPC // 2           # 2 example-pairs per core
FC = (H * 8) // 128         # 48 f-chunks
HH = H // 2

# data-dependent compile parameters (set by host_prep; defaults match the
# bundled fixed-seed inputs)
_LAST_C = 128               # packed triplet columns per pair (<= 128)
_LAST_K = 2                 # 128-row contraction chunks per example

GO = 8                      # f-chunks per transpose/FC group
MUL_CH = 4                  # f-chunks per elementwise-mul instruction
DVE_MULS = {0, 7}           # mul-instr indices (of 12/pair) on DVE


def build_program(reps=1, C=None, K=None):
    C = _LAST_C if C is None else C
    K = _LAST_K if K is None else K
    nc = bacc.Bacc("TRN2", target_bir_lowering=False, debug=False)

    KK = 2 * K              # contraction chunks per pair
    hid_d = nc.dram_tensor("hid", (NPAIR * KK * 128, H), BF16, kind="ExternalInput")
    at_d = nc.dram_tensor("at", (NPAIR * KK * 128, 2 * E), BF16, kind="ExternalInput")
    gg_d = nc.dram_tensor("gg", (NPAIR, 128, 3 * C), BF16, kind="ExternalInput")
    # W pre-shuffled on host to the SBUF chunk layout [p, (c n)]
    w_d = nc.dram_tensor("w", (128, FC * NT), BF16, kind="ExternalInput")
    out_d = nc.dram_tensor("out", (NT, NPAIR, C), F32, kind="ExternalOutput")

    with tile.TileContext(nc) as tc:
        with (
            tc.tile_pool(name="consts", bufs=1) as consts,
            tc.tile_pool(name="hidp", bufs=2 * KK) as hidp,
            tc.tile_pool(name="atp", bufs=2) as atp,
            tc.tile_pool(name="entsb", bufs=1) as entsb,
            tc.tile_pool(name="gps", bufs=3, space="PSUM") as gps,
            tc.tile_pool(name="trps", bufs=3, space="PSUM") as trps,
            tc.tile_pool(name="lgps", bufs=1, space="PSUM") as lgps,
            tc.tile_pool(name="tsb", bufs=1) as tsb,
            tc.tile_pool(name="blp", bufs=12) as blp,
            tc.tile_pool(name="blsb", bufs=3) as blsb,
            tc.tile_pool(name="outp", bufs=1) as outp,
        ):
          for _rep in range(reps):
            # ---- input DMAs (pair 0's stage-1 inputs first)
            hid_t = [None] * NPAIR   # list of KK tiles [128, H] per pair
            at_t = [None] * NPAIR    # [128, KK, 2E]
            gs_t = [None] * NPAIR
            go_t = [None] * NPAIR
            hid_r = hid_d[:].rearrange("(pk p) h -> p pk h", p=128)
            at_r = at_d[:].rearrange("(pk p) e -> p pk e", p=128)
            for P in range(NPAIR):
                att = atp.tile([128, KK, 2 * E], BF16)
                nc.sync.dma_start(att[:], at_r[:, P * KK : (P + 1) * KK, :])
                at_t[P] = att
                hts = []
                for ck in range(KK):
                    h1 = hidp.tile([128, H], BF16)
                    nc.sync.dma_start(h1[:], hid_r[:, P * KK + ck, :])
                    hts.append(h1)
                hid_t[P] = hts
                gg = consts.tile([128, 3 * C], BF16, tag=f"gg{P}")
                nc.sync.dma_start(gg[:], gg_d[P])
                gs_t[P] = gg[:][:, :C]
                go_t[P] = gg[:][:, C : 2 * C]
                if P == 0:
                    id_t = gg[:][:C, 2 * C :]
                    w_all = consts.tile([128, FC, NT], BF16, tag="w")
                    nc.sync.dma_start(
                        w_all[:], w_d[:].rearrange("p (c n) -> p c n", n=NT))

            # ---- stage 1 for both pairs first (ent PSUM borrows a gps tile)
            ent_sb = []
            for P in range(NPAIR):
                esb = entsb.tile([128, H], BF16, tag=f"ent{P}")
                for nh in range(2):
                    ent_tile = gps.tile([128, 512], F32, tag="g")
                    ent_ps = ent_tile[:][:, :HH]
                    for ck in range(KK):
                        nc.tensor.matmul(
                            ent_ps,
                            at_t[P][:, ck, :],
                            hid_t[P][ck][:, nh * HH : (nh + 1) * HH],
                            start=(ck == 0),
                            stop=(ck == KK - 1),
                        )
                    if nh == 0:
                        nc.scalar.copy(esb[:, :HH], ent_ps)
                    else:
                        nc.vector.tensor_copy(esb[:, HH:], ent_ps)
                ent_sb.append(esb)

            # ---- stage 2: gathers for both pairs, then muls, then
            # transpose+FC — keeps PE fed while the first muls run
            lg = lgps.tile([NT, NPAIR, C], F32)
            st_sb, ot_sb = [], []
            for P in range(NPAIR):
                esb = ent_sb[P]
                ss = tsb.tile([128, H], BF16, tag=f"sT{P}")
                oo = tsb.tile([128, H], BF16, tag=f"oT{P}")
                st_sb.append(ss)
                ot_sb.append(oo)
                for side, gmat, dst in ((0, gs_t[P], ss), (1, go_t[P], oo)):
                    for nh in range(2):
                        t_tile = gps.tile([128, 512], F32, tag="g")
                        t_ps = t_tile[:][:C, :HH]
                        nc.tensor.matmul(
                            t_ps, gmat,
                            esb[:, nh * HH : (nh + 1) * HH],
                            start=True, stop=True)
                        if side == 0:
                            nc.scalar.copy(dst[:C, nh * HH : (nh + 1) * HH], t_ps)
                        else:
                            nc.vector.tensor_copy(
                                dst[:C, nh * HH : (nh + 1) * HH], t_ps)

            # bl_t[t, (c, gg, i, j)] muls, split DVE / GPSIMD; one bl
            # tile per transpose group so deps stay group-granular
            blts = [[] for _ in range(NPAIR)]
            for P in range(NPAIR):
                s_r = st_sb[P][:C].rearrange("p (g i) -> p g i", i=8)
                o_r = ot_sb[P][:C].rearrange("p (g j) -> p g j", j=8)
                for g in range(FC // GO):
                    blt = blp.tile([128, GO, 128], BF16)
                    blts[P].append(blt)
                    for mh in range(GO // MUL_CH):
                        m = g * (GO // MUL_CH) + mh
                        c0 = g * GO + mh * MUL_CH
                        c1 = c0 + MUL_CH
                        g0, g1_ = 2 * c0, 2 * c1
                        s_op = (s_r[:, g0:g1_, :].unsqueeze(3)
                                .broadcast_to((C, g1_ - g0, 8, 8)))
                        o_op = (o_r[:, g0:g1_, :].unsqueeze(2)
                                .broadcast_to((C, g1_ - g0, 8, 8)))
                        dst = blt[:C, mh * MUL_CH : (mh + 1) * MUL_CH, :].rearrange(
                            "p c (gg i j) -> p (c gg) i j", gg=2, i=8)
                        if m in DVE_MULS:
                            nc.vector.tensor_mul(dst, s_op, o_op)
                        else:
                            nc.gpsimd.tensor_mul(dst, s_op, o_op)

            # transpose + evac + FC per 8-chunk group; one group per pair
            # goes through the DMA XBAR transpose instead of PE+evac
            for P in range(NPAIR):
                for g in range(FC // GO):
                    blt = blts[P][g]
                    blT = blsb.tile([128, GO, C], BF16)
                    if g in (2, 5):
                        for j in range(GO):
                            nc.sync.dma_start(
                                blT[:, j, :C], blt[:C, j, :], transpose=True)
                    else:
                        trp = trps.tile([128, GO, C], BF16, tag="tr")
                        for j in range(GO):
                            nc.tensor.transpose(
                                trp[:, j, :C], blt[:C, j, :], id_t)
                        if g % 2 == 1:
                            nc.vector.tensor_copy(blT[:], trp[:])
                        else:
                            nc.scalar.copy(blT[:], trp[:])
                    for j in range(GO):
                        c = g * GO + j
                        nc.tensor.matmul(
                            lg[:, P, :],
                            w_all[:, c, :],
                            blT[:, j, :],
                            start=(c == 0),
                            stop=(c == FC - 1),
                        )

            out_sb = outp.tile([NT, NPAIR, C], F32)
            nc.scalar.copy(out_sb[:], lg[:])
            nc.sync.dma_start(out_d[:], out_sb[:])

    nc.compile()
    return nc


def _pair_examples(nv):
    """Pair up examples to minimize the max pair sum (greedy fold + local
    search over pairwise re-pairings)."""
    nv = np.asarray(nv)
    order = list(np.argsort(-nv))
    n = len(order) // 2
    pairs = [[order[i], order[2 * n - 1 - i]] for i in range(n)]

    def ps(p):
        return int(nv[p[0]] + nv[p[1]])

    changed = True
    it = 0
    while changed and it < 1000:
        changed = False
        it += 1
        for i in range(n):
            for j in range(i + 1, n):
                a, b = pairs[i], pairs[j]
                cur = max(ps(a), ps(b))
                for (x, y) in (((a[0], b[0]), (a[1], b[1])),
                               ((a[0], b[1]), (a[1], b[0]))):
                    m = max(int(nv[x[0]] + nv[x[1]]), int(nv[y[0]] + nv[y[1]]))
                    if m < cur:
                        pairs[i], pairs[j] = list(x), list(y)
                        a, b = pairs[i], pairs[j]
                        cur = m
                        changed = True
    return pairs


# layout metadata shared between host_prep / assemble / kernel
_LAYOUT = None      # per pair: (bs, ts) arrays for packed columns
_SPILL = None       # list of (b, t, logits_row) computed on host


def host_prep(hidden_states, entity_subw_indices, entity_subw_mask,
              triplet_entity_nums, pair_mask, W_fc):
    """Build per-core input maps (numpy only, cheap)."""
    global _LAST_C, _LAST_K, _LAYOUT, _SPILL
    import ml_dtypes
    bf16 = ml_dtypes.bfloat16
    hs = np.asarray(hidden_states, dtype=np.float32)
    idx = np.asarray(entity_subw_indices)
    msk = np.asarray(entity_subw_mask).astype(np.float32)
    trip = np.asarray(triplet_entity_nums)
    pm = np.asarray(pair_mask)
    # shuffle W to the SBUF chunk layout [p, (c, n)]
    w = (np.asarray(W_fc, dtype=np.float32).reshape(FC, 128, NT)
         .transpose(1, 0, 2).reshape(128, FC * NT).astype(bf16))

    cnt = np.maximum(msk.sum(axis=2), 1.0)          # (B, E)
    wgt = msk / cnt[:, :, None]                     # (B, E, S)

    # distinct hidden rows per example
    used = [np.unique(idx[b][msk[b] > 0]) for b in range(B)]
    K = max(1, int(np.ceil(max(len(u) for u in used) / 128)))
    KR = K * 128
    hidg = np.zeros((B, KR, H), bf16)
    at2 = np.zeros((B, KR, E), np.float32)
    for b in range(B):
        u = used[b]
        hidg[b, : len(u)] = hs[b, u].astype(bf16)
        pos = np.full(L, -1, np.int64)
        pos[u] = np.arange(len(u))
        e_i, s_i = np.nonzero(msk[b] > 0)
        np.add.at(at2[b], (pos[idx[b, e_i, s_i]], e_i), wgt[b, e_i, s_i])
    at2 = at2.astype(bf16)

    nv = pm.sum(axis=1).astype(np.int64)
    pairs = _pair_examples(nv)
    maxsum = max(int(nv[a] + nv[b]) for a, b in pairs)
    C = min(128, int(np.ceil(max(maxsum, 4) / 4) * 4))
    _LAST_C, _LAST_K = C, K

    KK = 2 * K
    gg = np.zeros((len(pairs), 128, 3 * C), bf16)
    gs = gg[:, :, :C]
    go = gg[:, :, C : 2 * C]
    for p in range(len(pairs)):
        gg[p, :C, 2 * C :] = np.eye(C, dtype=bf16)
    at_pair = np.zeros((len(pairs), KK * 128, 2 * E), bf16)
    hid_pair = np.zeros((len(pairs), KK * 128, H), bf16)
    colmap = []  # per pair: (bs array, ts array)
    spill = []
    for p, (bx, by) in enumerate(pairs):
        hid_pair[p, :KR] = hidg[bx]
        hid_pair[p, KR:] = hidg[by]
        at_pair[p, :KR, :E] = at2[bx]
        at_pair[p, KR:, E:] = at2[by]
        bs, ts = [], []
        k = 0
        for exl, b in ((0, bx), (1, by)):
            tv = np.nonzero(pm[b])[0]
            keep = min(len(tv), C - k)
            for t in tv[keep:]:
                spill.append((b, int(t)))
            tv = tv[:keep]
            n = len(tv)
            gs[p, exl * E + trip[b, tv, 0], k + np.arange(n)] = 1.0
            go[p, exl * E + trip[b, tv, 1], k + np.arange(n)] = 1.0
            bs.append(np.full(n, b))
            ts.append(tv)
            k += n
        colmap.append((np.concatenate(bs), np.concatenate(ts)))
    _LAYOUT = colmap
    _SPILL = [(b, t,
               _host_logits_row(hs, idx, wgt, trip, b, t, W_fc))
              for b, t in spill]

    in_maps = []
    for c in range(NCORES):
        p0 = c * NPAIR
        in_maps.append({
            "hid": np.ascontiguousarray(
                hid_pair[p0 : p0 + NPAIR].reshape(NPAIR * KK * 128, H)),
            "at": np.ascontiguousarray(
                at_pair[p0 : p0 + NPAIR].reshape(NPAIR * KK * 128, 2 * E)),
            "gg": np.ascontiguousarray(gg[p0 : p0 + NPAIR]),
            "w": w,
        })
    return in_maps


def _host_logits_row(hs, idx, wgt, trip, b, t, W_fc):
    """Exact logits (without bias) for one (b, t) triplet."""
    e1, e2 = int(trip[b, t, 0]), int(trip[b, t, 1])
    subj = (wgt[b, e1][:, None] * hs[b, idx[b, e1]]).sum(0)
    obj = (wgt[b, e2][:, None] * hs[b, idx[b, e2]]).sum(0)
    bl = (subj.reshape(96, 8, 1) * obj.reshape(96, 1, 8)).reshape(-1)
    return bl @ np.asarray(W_fc, np.float32)


def assemble(results, b_fc):
    """results[c]["out"] is (NT, NPAIR, C) -> (B, T, NT) + bias."""
    bfc = np.asarray(b_fc, np.float32)
    logits = np.broadcast_to(bfc, (B, T, NT)).copy()
    for c in range(NCORES):
        o = np.asarray(results[c]["out"], np.float32)
        for P in range(NPAIR):
            bs, ts = _LAYOUT[c * NPAIR + P]
            n = len(bs)
            logits[bs, ts, :] = o[:, P, :n].T + bfc
    for b, t, row in _SPILL:
        logits[b, t, :] = row + bfc
    return logits


_NC_CACHE = {}


def kernel(hidden_states, entity_subw_indices, entity_subw_mask,
           triplet_entity_nums, pair_mask, W_fc, b_fc):
    in_maps = host_prep(hidden_states, entity_subw_indices, entity_subw_mask,
                        triplet_entity_nums, pair_mask, W_fc)
    key = (_LAST_C, _LAST_K)
    if key not in _NC_CACHE:
        _NC_CACHE[key] = build_program()
    nc = _NC_CACHE[key]
    res = run_bass_kernel_spmd(nc, in_maps, core_ids=list(range(NCORES)))
    return assemble(res.results, b_fc)



# revision 13
# speedup vs baseline: 3.4143x; 3.4143x over previous
"""Trainium2 Bass kernel for BilinearClassification (segment_reduce).

Math (per example b):
  ent[e,:]  = masked-mean over subword span of hidden[idx[e,s],:]      (E=64, H=768)
  subj[t,:] = ent[trip[t,0],:] * pm[t];  obj[t,:] = ent[trip[t,1],:] * pm[t]
  bl[t, (g,i,j)] = subj[t, g*8+i] * obj[t, g*8+j]                      (f = 6144)
  logits[t,n] = bl[t,:] @ W[:,n] + b[n]                                (NT=42)

Device strategy (8 cores, 4 examples = 2 example-pairs each, no collectives):
  - Only VALID triplets (pair_mask) are computed, packed into C<=128 columns
    per example-pair; the few overflow triplets are computed on the host.
  - Host gathers only the DISTINCT hidden rows each example references and
    PERMUTES hidden columns to i-major order: col' (i,g) = col (g*8+i).
    In i-major order the bilinear outer product bl[t, slot(i,j,g)] =
    s'[t, i*96+g] * o'[t, j*96+g] has unit-stride innermost dim (g) on
    BOTH operands and the destination -> DVE runs it in 2x perf mode with
    ONE instruction per (pair, i)-slice.  W rows are permuted to match.
  - stage 1: ent'[(ex,e), h'] = AT.T @ hidg per pair (PE).
  - gathers: s' = Gs.T @ ent', o' = Go.T @ ent'  (PE, one-hot matrices).
  - muls: bl_i[t, (j, g)] on DVE (some slices on GPSIMD).
  - bridge to [f, t]: per (pair, i) 6 PE transposes + evac, or one grouped
    XBAR DMA transpose ([C, 768] -> [128, 6, C]).
  - FC: logits[n, t] += W_c.T @ blT_c, 48 chunks of N=2C (both pairs in one
    moving operand), col-tiled 2x on the PE array (even chunks -> partitions
    0:42, odd chunks -> 64:106); halves summed on host.
  - host scatters packed columns back to (b, t) and adds b_fc.
"""
import sys

sys.path.insert(0, "/opt/trn_rl_repo")

import numpy as np

import concourse.bass as bass
import concourse.bacc as bacc
import concourse.tile as tile
from concourse import mybir
from concourse.bass_utils import run_bass_kernel_spmd

F32 = mybir.dt.float32
BF16 = mybir.dt.bfloat16

B, L, H = 32, 512, 768
E, S, T = 64, 8, 128
NT = 42
NCORES = 8
EXPC = B // NCORES          # 4 examples per core
NPAIR = EXPC // 2           # 2 example-pairs per core
FC = (H * 8) // 128         # 48 f-chunks
HH = H // 2
NI = 8                      # i-slices per pair (each 6 f-chunks)
NCH = 6                     # f-chunks per i-slice

# data-dependent compile parameters (set by host_prep; defaults match the
# bundled fixed-seed inputs)
_LAST_C = 128               # packed triplet columns per pair (<= 128)
_LAST_K = 2                 # 128-row contraction chunks per example

# (pair, i) units routed to the XBAR DMA transpose instead of PE+evac
DMA_TR = {(0, 1), (0, 5), (1, 3), (1, 7)}
# (pair, i) mul units routed to GPSIMD instead of DVE
GPS_MULS = {(0, 6), (1, 2)}
# (pair, i) units whose transpose-evac runs on DVE (rest: ACT)
DVE_EVAC = {(0, 0), (0, 4), (1, 1), (1, 5)}

# hidden-column permutation to i-major: hid'[:, i*96+g] = hid[:, g*8+i]
PERMH = np.arange(H).reshape(96, 8).T.reshape(-1)
# W-row permutation to slot order: slot(i,j,g) = i*768 + j*96 + g,
# original f = g*64 + i*8 + j
PERMW = np.arange(H * 8).reshape(96, 8, 8).transpose(1, 2, 0).reshape(-1)


def build_program(reps=1, C=None, K=None):
    C = _LAST_C if C is None else C
    K = _LAST_K if K is None else K
    nc = bacc.Bacc("TRN2", target_bir_lowering=False, debug=False)

    KK = 2 * K              # contraction chunks per pair
    hid_d = nc.dram_tensor("hid", (NPAIR * KK * 128, H), BF16, kind="ExternalInput")
    at_d = nc.dram_tensor("at", (NPAIR * KK * 128, 2 * E), BF16, kind="ExternalInput")
    gg_d = nc.dram_tensor("gg", (NPAIR, 128, 3 * C), BF16, kind="ExternalInput")
    # W pre-permuted (PERMW) and pre-shuffled to the SBUF chunk layout
    w_d = nc.dram_tensor("w", (128, FC * NT), BF16, kind="ExternalInput")
    # halves: [0] = even-chunk half, [1] = odd-chunk half (host sums)
    out_d = nc.dram_tensor("out", (2, NT, NPAIR, C), F32, kind="ExternalOutput")

    with tile.TileContext(nc) as tc:
        with (
            tc.tile_pool(name="consts", bufs=1) as consts,
            tc.tile_pool(name="hidp", bufs=2) as hidp,
            tc.tile_pool(name="entp", bufs=2) as entp,
            tc.tile_pool(name="sop", bufs=4) as sop,
            tc.tile_pool(name="gps", bufs=3, space="PSUM") as gps,
            tc.tile_pool(name="trps", bufs=3, space="PSUM") as trps,
            tc.tile_pool(name="lgps", bufs=1, space="PSUM") as lgps,
            tc.tile_pool(name="blp", bufs=6) as blp,
            tc.tile_pool(name="blsb", bufs=4) as blsb,
            tc.tile_pool(name="outp", bufs=1) as outp,
        ):
          hid_r = hid_d[:].rearrange("(pk p) h -> p pk h", p=128)
          at_r = at_d[:].rearrange("(pk p) e -> p pk e", p=128)
          gg_r = gg_d[:].rearrange("q p e -> p q e")
          for _rep in range(reps):
            # ---- input DMAs
            att = consts.tile([128, NPAIR, KK, 2 * E], BF16, tag="at")
            nc.sync.dma_start(
                att[:], at_r.rearrange("p (q k) e -> p q k e", q=NPAIR))
            gg = consts.tile([128, NPAIR, 3 * C], BF16, tag="gg")
            nc.sync.dma_start(gg[:], gg_r)
            gs_t = [gg[:][:, P, :C] for P in range(NPAIR)]
            go_t = [gg[:][:, P, C : 2 * C] for P in range(NPAIR)]
            id_t = gg[:][:C, 0, 2 * C :]
            w_all = consts.tile([128, FC, NT], BF16, tag="w")
            nc.sync.dma_start(w_all[:], w_d[:].rearrange("p (c n) -> p c n", n=NT))
            hid_t = []
            for P in range(NPAIR):
                ht = hidp.tile([128, KK, H], BF16)
                nc.sync.dma_start(ht[:], hid_r[:, P * KK : (P + 1) * KK, :])
                hid_t.append(ht)

            # ---- stage 1: ent'[(ex,e), h'] per pair
            ent_sb = []
            for P in range(NPAIR):
                esb = entp.tile([128, H], BF16, tag=f"ent{P}")
                for nh in range(2):
                    ps = gps.tile([128, 512], F32, tag="g")
                    ent_ps = ps[:][:, :HH]
                    for ck in range(KK):
                        nc.tensor.matmul(
                            ent_ps,
                            att[:][:, P, ck, :],
                            hid_t[P][:, ck, nh * HH : (nh + 1) * HH],
                            start=(ck == 0),
                            stop=(ck == KK - 1),
                        )
                    if nh == 0:
                        nc.scalar.copy(esb[:, :HH], ent_ps)
                    else:
                        nc.vector.tensor_copy(esb[:, HH:], ent_ps)
                ent_sb.append(esb)

            # ---- gathers: o' first (muls need full o'), then s'
            st_sb, ot_sb = [None] * NPAIR, [None] * NPAIR
            for P in range(NPAIR):
                oo = sop.tile([128, H], BF16, tag=f"oT{P}")
                ss = sop.tile([128, H], BF16, tag=f"sT{P}")
                ot_sb[P], st_sb[P] = oo, ss
                for gmat, dst in ((go_t[P], oo), (gs_t[P], ss)):
                    for nh in range(2):
                        ps = gps.tile([128, 512], F32, tag="g")
                        t_ps = ps[:][:C, :HH]
                        nc.tensor.matmul(
                            t_ps, gmat,
                            ent_sb[P][:, nh * HH : (nh + 1) * HH],
                            start=True, stop=True)
                        if nh == 0:
                            nc.scalar.copy(dst[:C, :HH], t_ps)
                        else:
                            nc.vector.tensor_copy(dst[:C, HH:], t_ps)

            # ---- muls + transposes + FC, pipelined per i-slice
            lgA = lgps.tile([128, 512], F32, tag="lgA")
            lgB = lgps.tile([128, 512], F32, tag="lgB")
            for i in range(NI):
                blT = blsb.tile([128, NCH, NPAIR, C], BF16)
                for P in range(NPAIR):
                    # bl_i[t, (j, g)] = s'[t, i*96+g] * o'[t, j*96+g]
                    bl = blp.tile([128, NI, 96], BF16)
                    s_op = (st_sb[P][:C, i * 96 : (i + 1) * 96]
                            .unsqueeze(1).broadcast_to((C, 8, 96)))
                    o_op = ot_sb[P][:C].rearrange("p (j g) -> p j g", g=96)
                    dst = bl[:C]
                    if (P, i) in GPS_MULS:
                        nc.gpsimd.tensor_mul(dst, s_op, o_op)
                    else:
                        nc.vector.tensor_mul(dst, s_op, o_op)
                    bl2 = bl[:C].rearrange("p a b -> p (a b)")
                    if (P, i) in DMA_TR:
                        nc.sync.dma_start(
                            blT[:][:, :, P, :], bl2, transpose=True)
                    else:
                        trp = trps.tile([128, NCH, C], BF16, tag="tr")
                        for m in range(NCH):
                            nc.tensor.transpose(
                                trp[:][:, m, :C],
                                bl2[:, m * 128 : (m + 1) * 128], id_t)
                        if (P, i) in DVE_EVAC:
                            nc.vector.tensor_copy(blT[:][:, :, P, :], trp[:])
                        else:
                            nc.scalar.copy(blT[:][:, :, P, :], trp[:])
                # FC for this i-slice: 6 chunks, both pairs per matmul
                for m in range(NCH):
                    q = i * NCH + m
                    half, qq = q % 2, q // 2
                    out_ap = (lgA[:][0:42, : NPAIR * C] if half == 0
                              else lgB[:][64:106, : NPAIR * C])
                    nc.tensor.matmul(
                        out_ap,
                        w_all[:][:, q, :],
                        blT[:][:, m, :, :],
                        start=(qq == 0),
                        stop=(qq == 23),
                    )

            out_sb = outp.tile([128, NPAIR, C], F32)
            nc.vector.tensor_copy(
                out_sb[:][0:42].rearrange("p q c -> p (q c)"),
                lgA[:][0:42, : NPAIR * C])
            nc.vector.tensor_copy(
                out_sb[:][64:106].rearrange("p q c -> p (q c)"),
                lgB[:][64:106, : NPAIR * C])
            nc.sync.dma_start(out_d[0], out_sb[:][0:NT])
            nc.sync.dma_start(out_d[1], out_sb[:][64 : 64 + NT])

    nc.compile()
    return nc


def _pair_examples(nv):
    """Pair up examples to minimize the max pair sum (greedy fold + local
    search over pairwise re-pairings)."""
    nv = np.asarray(nv)
    order = list(np.argsort(-nv))
    n = len(order) // 2
    pairs = [[order[i], order[2 * n - 1 - i]] for i in range(n)]

    def ps(p):
        return int(nv[p[0]] + nv[p[1]])

    changed = True
    it = 0
    while changed and it < 1000:
        changed = False
        it += 1
        for i in range(n):
            for j in range(i + 1, n):
                a, b = pairs[i], pairs[j]
                cur = max(ps(a), ps(b))
                for (x, y) in (((a[0], b[0]), (a[1], b[1])),
                               ((a[0], b[1]), (a[1], b[0]))):
                    m = max(int(nv[x[0]] + nv[x[1]]), int(nv[y[0]] + nv[y[1]]))
                    if m < cur:
                        pairs[i], pairs[j] = list(x), list(y)
                        a, b = pairs[i], pairs[j]
                        cur = m
                        changed = True
    return pairs


# layout metadata shared between host_prep / assemble / kernel
_LAYOUT = None      # per pair: (bs, ts) arrays for packed columns
_SPILL = None       # list of (b, t, logits_row) computed on host


def host_prep(hidden_states, entity_subw_indices, entity_subw_mask,
              triplet_entity_nums, pair_mask, W_fc):
    """Build per-core input maps (numpy only, cheap)."""
    global _LAST_C, _LAST_K, _LAYOUT, _SPILL
    import ml_dtypes
    bf16 = ml_dtypes.bfloat16
    hs = np.asarray(hidden_states, dtype=np.float32)
    idx = np.asarray(entity_subw_indices)
    msk = np.asarray(entity_subw_mask).astype(np.float32)
    trip = np.asarray(triplet_entity_nums)
    pm = np.asarray(pair_mask)
    # permute W rows to slot order, then shuffle to the SBUF chunk layout
    w = (np.asarray(W_fc, dtype=np.float32)[PERMW].reshape(FC, 128, NT)
         .transpose(1, 0, 2).reshape(128, FC * NT).astype(bf16))

    cnt = np.maximum(msk.sum(axis=2), 1.0)          # (B, E)
    wgt = msk / cnt[:, :, None]                     # (B, E, S)

    # distinct hidden rows per example; columns permuted to i-major
    used = [np.unique(idx[b][msk[b] > 0]) for b in range(B)]
    K = max(1, int(np.ceil(max(len(u) for u in used) / 128)))
    KR = K * 128
    hidg = np.zeros((B, KR, H), bf16)
    at2 = np.zeros((B, KR, E), np.float32)
    for b in range(B):
        u = used[b]
        hidg[b, : len(u)] = hs[b][u][:, PERMH].astype(bf16)
        pos = np.full(L, -1, np.int64)
        pos[u] = np.arange(len(u))
        e_i, s_i = np.nonzero(msk[b] > 0)
        np.add.at(at2[b], (pos[idx[b, e_i, s_i]], e_i), wgt[b, e_i, s_i])
    at2 = at2.astype(bf16)

    nv = pm.sum(axis=1).astype(np.int64)
    pairs = _pair_examples(nv)
    maxsum = max(int(nv[a] + nv[b]) for a, b in pairs)
    C = min(128, int(np.ceil(max(maxsum, 16) / 16) * 16))
    _LAST_C, _LAST_K = C, K

    KK = 2 * K
    gg = np.zeros((len(pairs), 128, 3 * C), bf16)
    gs = gg[:, :, :C]
    go = gg[:, :, C : 2 * C]
    gg[0, :C, 2 * C :] = np.eye(C, dtype=bf16)
    for p in range(1, len(pairs)):
        gg[p, :C, 2 * C :] = np.eye(C, dtype=bf16)
    at_pair = np.zeros((len(pairs), KK * 128, 2 * E), bf16)
    hid_pair = np.zeros((len(pairs), KK * 128, H), bf16)
    colmap = []  # per pair: (bs array, ts array)
    spill = []
    for p, (bx, by) in enumerate(pairs):
        hid_pair[p, :KR] = hidg[bx]
        hid_pair[p, KR:] = hidg[by]
        at_pair[p, :KR, :E] = at2[bx]
        at_pair[p, KR:, E:] = at2[by]
        bs, ts = [], []
        k = 0
        for exl, b in ((0, bx), (1, by)):
            tv = np.nonzero(pm[b])[0]
            keep = min(len(tv), C - k)
            for t in tv[keep:]:
                spill.append((b, int(t)))
            tv = tv[:keep]
            n = len(tv)
            gs[p, exl * E + trip[b, tv, 0], k + np.arange(n)] = 1.0
            go[p, exl * E + trip[b, tv, 1], k + np.arange(n)] = 1.0
            bs.append(np.full(n, b))
            ts.append(tv)
            k += n
        colmap.append((np.concatenate(bs), np.concatenate(ts)))
    _LAYOUT = colmap
    _SPILL = [(b, t,
               _host_logits_row(hs, idx, wgt, trip, b, t, W_fc))
              for b, t in spill]

    in_maps = []
    for c in range(NCORES):
        p0 = c * NPAIR
        in_maps.append({
            "hid": np.ascontiguousarray(
                hid_pair[p0 : p0 + NPAIR].reshape(NPAIR * KK * 128, H)),
            "at": np.ascontiguousarray(
                at_pair[p0 : p0 + NPAIR].reshape(NPAIR * KK * 128, 2 * E)),
            "gg": np.ascontiguousarray(gg[p0 : p0 + NPAIR]),
            "w": w,
        })
    return in_maps


def _host_logits_row(hs, idx, wgt, trip, b, t, W_fc):
    """Exact logits (without bias) for one (b, t) triplet."""
    e1, e2 = int(trip[b, t, 0]), int(trip[b, t, 1])
    subj = (wgt[b, e1][:, None] * hs[b, idx[b, e1]]).sum(0)
    obj = (wgt[b, e2][:, None] * hs[b, idx[b, e2]]).sum(0)
    bl = (subj.reshape(96, 8, 1) * obj.reshape(96, 1, 8)).reshape(-1)
    return bl @ np.asarray(W_fc, np.float32)


def assemble(results, b_fc):
    """results[c]["out"] is (2, NT, NPAIR, C) -> (B, T, NT) + bias."""
    bfc = np.asarray(b_fc, np.float32)
    logits = np.broadcast_to(bfc, (B, T, NT)).copy()
    for c in range(NCORES):
        o = np.asarray(results[c]["out"], np.float32)
        ologit = o[0] + o[1]                        # (NT, NPAIR, C)
        for P in range(NPAIR):
            bs, ts = _LAYOUT[c * NPAIR + P]
            n = len(bs)
            logits[bs, ts, :] = ologit[:, P, :n].T + bfc
    for b, t, row in _SPILL:
        logits[b, t, :] = row + bfc
    return logits


_NC_CACHE = {}


def kernel(hidden_states, entity_subw_indices, entity_subw_mask,
           triplet_entity_nums, pair_mask, W_fc, b_fc):
    in_maps = host_prep(hidden_states, entity_subw_indices, entity_subw_mask,
                        triplet_entity_nums, pair_mask, W_fc)
    key = (_LAST_C, _LAST_K)
    if key not in _NC_CACHE:
        _NC_CACHE[key] = build_program()
    nc = _NC_CACHE[key]
    res = run_bass_kernel_spmd(nc, in_maps, core_ids=list(range(NCORES)))
    return assemble(res.results, b_fc)


# revision 25
# speedup vs baseline: 4.2190x; 1.2357x over previous
"""Trainium2 Bass kernel for BilinearClassification (segment_reduce).

Math (per example b):
  ent[e,:]  = masked-mean over subword span of hidden[idx[e,s],:]      (E=64, H=768)
  subj[t,:] = ent[trip[t,0],:] * pm[t];  obj[t,:] = ent[trip[t,1],:] * pm[t]
  bl[t, (g,i,j)] = subj[t, g*8+i] * obj[t, g*8+j]                      (f = 6144)
  logits[t,n] = bl[t,:] @ W[:,n] + b[n]                                (NT=42)

Device strategy (8 cores, 4 examples = 2 example-pairs each, no collectives):
  - Only VALID triplets (pair_mask) are computed, packed into C<=128 columns
    per example-pair; the few overflow triplets are computed on the host.
  - Host gathers only the DISTINCT hidden rows each example references and
    PERMUTES hidden columns to i-major order: col' (i,g) = col (g*8+i).
    In i-major order the bilinear outer product bl[t, slot(i,j,g)] =
    s'[t, i*96+g] * o'[t, j*96+g] has unit-stride innermost dim (g) on
    BOTH operands and the destination -> DVE runs it in 2x perf mode with
    ONE instruction per (pair, i)-slice.  W rows are permuted to match.
  - stage 1: ent'[(ex,e), h'] = AT.T @ hidg per pair (PE).
  - gathers: s' = Gs.T @ ent', o' = Go.T @ ent'  (PE, one-hot matrices).
  - muls: bl_i[t, (j, g)] on DVE (some slices on GPSIMD).
  - bridge to [f, t]: per (pair, i) 6 PE transposes + evac, or one grouped
    XBAR DMA transpose ([C, 768] -> [128, 6, C]).
  - FC: logits[n, t] += W_c.T @ blT_c, 48 chunks of N=2C (both pairs in one
    moving operand), col-tiled 2x on the PE array (even chunks -> partitions
    0:42, odd chunks -> 64:106); halves summed on host.
  - host scatters packed columns back to (b, t) and adds b_fc.
"""
import sys

sys.path.insert(0, "/opt/trn_rl_repo")

import numpy as np

import concourse.bass as bass
import concourse.bacc as bacc
import concourse.tile as tile
from concourse import mybir
from concourse.bass_utils import run_bass_kernel_spmd

F32 = mybir.dt.float32
BF16 = mybir.dt.bfloat16

B, L, H = 32, 512, 768
E, S, T = 64, 8, 128
NT = 42
NCORES = 8
EXPC = B // NCORES          # 4 examples per core
NPAIR = EXPC // 2           # 2 example-pairs per core
FC = (H * 8) // 128         # 48 f-chunks
HH = H // 2
NI = 8                      # i-slices per pair (each 6 f-chunks)
NCH = 6                     # f-chunks per i-slice

# data-dependent compile parameters (set by host_prep; defaults match the
# bundled fixed-seed inputs)
_LAST_C = 128               # packed triplet columns per pair (<= 128)
_LAST_K = 2                 # 128-row contraction chunks per example

# (pair, i) units routed to the XBAR DMA transpose instead of PE+evac
# (early i so the longer mul->HWDGE->xbar chain hides under the pipeline)
DMA_TR = {(0, 0), (0, 2), (0, 4), (1, 0), (1, 2), (1, 4)}
# (pair, i) mul units routed to GPSIMD instead of DVE (early: GPS is slow)
GPS_MULS = {(0, 1), (1, 1), (0, 3), (1, 3)}
# (pair, i) units whose transpose-evac runs on DVE (rest: ACT)
DVE_EVAC = {(0, 1), (0, 5), (1, 3), (1, 7), (0, 7)}

# hidden-column permutation to i-major: hid'[:, i*96+g] = hid[:, g*8+i]
PERMH = np.arange(H).reshape(96, 8).T.reshape(-1)
# W-row permutation to slot order: slot(i,j,g) = i*768 + j*96 + g,
# original f = g*64 + i*8 + j
PERMW = np.arange(H * 8).reshape(96, 8, 8).transpose(1, 2, 0).reshape(-1)


def build_program(reps=1, C=None, K=None):
    C = _LAST_C if C is None else C
    K = _LAST_K if K is None else K
    nc = bacc.Bacc("TRN2", target_bir_lowering=False, debug=False)

    KK = 2 * K              # contraction chunks per pair
    hid_d = nc.dram_tensor("hid", (NPAIR * KK * 128, H), BF16, kind="ExternalInput")
    at_d = nc.dram_tensor("at", (NPAIR * KK * 128, 2 * E), BF16, kind="ExternalInput")
    gg_d = nc.dram_tensor("gg", (NPAIR, 128, 3 * C), BF16, kind="ExternalInput")
    # W pre-permuted (PERMW) and pre-shuffled to the SBUF chunk layout
    w_d = nc.dram_tensor("w", (128, FC * NT), BF16, kind="ExternalInput")
    # halves: [0] = even-chunk half, [1] = odd-chunk half (host sums)
    out_d = nc.dram_tensor("out", (2, NT, NPAIR, C), F32, kind="ExternalOutput")

    with tile.TileContext(nc) as tc:
        with (
            tc.tile_pool(name="consts", bufs=2) as consts,
            tc.tile_pool(name="hidp", bufs=4) as hidp,
            tc.tile_pool(name="entp", bufs=4) as entp,
            tc.tile_pool(name="sop", bufs=8) as sop,
            tc.tile_pool(name="gps", bufs=3, space="PSUM") as gps,
            tc.tile_pool(name="trps", bufs=3, space="PSUM") as trps,
            tc.tile_pool(name="lgps", bufs=1, space="PSUM") as lgps,
            tc.tile_pool(name="blp", bufs=8) as blp,
            tc.tile_pool(name="blsb", bufs=2) as blsb,
            tc.tile_pool(name="outp", bufs=2) as outp,
        ):
          hid_r = hid_d[:].rearrange("(pk p) h -> p pk h", p=128)
          at_r = at_d[:].rearrange("(pk p) e -> p pk e", p=128)
          gg_r = gg_d[:].rearrange("q p e -> p q e")

          def emit_dmas():
            att = consts.tile([128, NPAIR, KK, 2 * E], BF16, tag="at")
            nc.sync.dma_start(
                att[:], at_r.rearrange("p (q k) e -> p q k e", q=NPAIR))
            gg = consts.tile([128, NPAIR, 3 * C], BF16, tag="gg")
            nc.sync.dma_start(gg[:], gg_r)
            w_all = consts.tile([128, FC, NT], BF16, tag="w")
            nc.sync.dma_start(w_all[:], w_d[:].rearrange("p (c n) -> p c n", n=NT))
            hid_t = []
            for P in range(NPAIR):
                ht = hidp.tile([128, KK, H], BF16)
                nc.sync.dma_start(ht[:], hid_r[:, P * KK : (P + 1) * KK, :])
                hid_t.append(ht)
            return att, gg, w_all, hid_t

          cur = emit_dmas()
          for _rep in range(reps):
            att, gg, w_all, hid_t = cur
            cur = emit_dmas() if _rep + 1 < reps else None
            gs_t = [gg[:][:, P, :C] for P in range(NPAIR)]
            go_t = [gg[:][:, P, C : 2 * C] for P in range(NPAIR)]
            id_t = gg[:][:C, 0, 2 * C :]

            # ---- per pair: stage1 -> gathers -> muls -> transposes; FC last
            lgA = lgps.tile([128, 512], F32, tag="lgA")
            lgB = lgps.tile([128, 512], F32, tag="lgB")
            blT = [blsb.tile([128, NCH, NPAIR, C], BF16, name=f"blT{_i}")
                   for _i in range(NI)]
            ss_l, oo_l = [], []
            for P in range(NPAIR):
                # stage 1: ent'[(ex,e), h']
                esb = entp.tile([128, H], BF16, tag=f"ent{P}")
                for nh in range(2):
                    ps = gps.tile([128, 512], F32, tag="g")
                    ent_ps = ps[:][:, :HH]
                    for ck in range(KK):
                        nc.tensor.matmul(
                            ent_ps,
                            att[:][:, P, ck, :],
                            hid_t[P][:, ck, nh * HH : (nh + 1) * HH],
                            start=(ck == 0),
                            stop=(ck == KK - 1),
                        )
                    if nh == 0:
                        nc.scalar.copy(esb[:, :HH], ent_ps)
                    else:
                        nc.scalar.copy(esb[:, HH:], ent_ps)
                # gathers: o' first (muls need full o'), then s'
                oo = sop.tile([128, H], BF16, tag=f"oT{P}")
                ss = sop.tile([128, H], BF16, tag=f"sT{P}")
                oo_l.append(oo)
                ss_l.append(ss)
                for gmat, dst in ((go_t[P], oo), (gs_t[P], ss)):
                    for nh in range(2):
                        ps = gps.tile([128, 512], F32, tag="g")
                        t_ps = ps[:][:C, :HH]
                        nc.tensor.matmul(
                            t_ps, gmat,
                            esb[:, nh * HH : (nh + 1) * HH],
                            start=True, stop=True)
                        if nh == 0:
                            nc.scalar.copy(dst[:C, :HH], t_ps)
                        elif dst is oo:
                            nc.scalar.copy(dst[:C, HH:], t_ps)
                        else:
                            nc.vector.tensor_copy(dst[:C, HH:], t_ps)
            # muls + transposes per (pair, i-slice)
            for P in range(NPAIR):
                ss, oo = ss_l[P], oo_l[P]
                for i in range(NI):
                    # bl_i[t, (j, g)] = s'[t, i*96+g] * o'[t, j*96+g]
                    bl = blp.tile([128, 8, 96], BF16)
                    s_op = (ss[:C, i * 96 : (i + 1) * 96]
                            .unsqueeze(1).broadcast_to((C, 8, 96)))
                    o_op = oo[:C].rearrange("p (j g) -> p j g", g=96)
                    dst = bl[:C]
                    if (P, i) in GPS_MULS:
                        nc.gpsimd.tensor_mul(dst, s_op, o_op)
                    else:
                        nc.vector.tensor_mul(dst, s_op, o_op)
                    bl2 = bl[:C].rearrange("p a b -> p (a b)")
                    if (P, i) in DMA_TR:
                        nc.sync.dma_start(
                            blT[i][:][:, :, P, :], bl2, transpose=True)
                    else:
                        trp = trps.tile([128, NCH, C], BF16, tag="tr")
                        for m in range(NCH):
                            nc.tensor.transpose(
                                trp[:][:, m, :C],
                                bl2[:, m * 128 : (m + 1) * 128], id_t)
                        if (P, i) in DVE_EVAC:
                            nc.vector.tensor_copy(
                                blT[i][:][:, :, P, :], trp[:])
                        else:
                            nc.scalar.copy(blT[i][:][:, :, P, :], trp[:])
            # FC: 6 chunks per i-slice, both pairs per matmul
            for i in range(NI):
                for m in range(NCH):
                    q = i * NCH + m
                    half, qq = q % 2, q // 2
                    out_ap = (lgA[:][0:42, : NPAIR * C] if half == 0
                              else lgB[:][64:106, : NPAIR * C])
                    nc.tensor.matmul(
                        out_ap,
                        w_all[:][:, q, :],
                        blT[i][:][:, m, :, :],
                        start=(qq == 0),
                        stop=(qq == 23),
                    )

            out_sb = outp.tile([128, NPAIR, C], F32)
            nc.scalar.copy(
                out_sb[:][0:42].rearrange("p q c -> p (q c)"),
                lgA[:][0:42, : NPAIR * C])
            nc.vector.tensor_copy(
                out_sb[:][64:106].rearrange("p q c -> p (q c)"),
                lgB[:][64:106, : NPAIR * C])
            nc.sync.dma_start(out_d[0], out_sb[:][0:NT])
            nc.sync.dma_start(out_d[1], out_sb[:][64 : 64 + NT])

    nc.compile()
    return nc


def _pair_examples(nv):
    """Pair up examples to minimize the max pair sum (greedy fold + local
    search over pairwise re-pairings)."""
    nv = np.asarray(nv)
    order = list(np.argsort(-nv))
    n = len(order) // 2
    pairs = [[order[i], order[2 * n - 1 - i]] for i in range(n)]

    def ps(p):
        return int(nv[p[0]] + nv[p[1]])

    changed = True
    it = 0
    while changed and it < 1000:
        changed = False
        it += 1
        for i in range(n):
            for j in range(i + 1, n):
                a, b = pairs[i], pairs[j]
                cur = max(ps(a), ps(b))
                for (x, y) in (((a[0], b[0]), (a[1], b[1])),
                               ((a[0], b[1]), (a[1], b[0]))):
                    m = max(int(nv[x[0]] + nv[x[1]]), int(nv[y[0]] + nv[y[1]]))
                    if m < cur:
                        pairs[i], pairs[j] = list(x), list(y)
                        a, b = pairs[i], pairs[j]
                        cur = m
                        changed = True
    return pairs


# layout metadata shared between host_prep / assemble / kernel
_LAYOUT = None      # per pair: (bs, ts) arrays for packed columns
_SPILL = None       # list of (b, t, logits_row) computed on host


def host_prep(hidden_states, entity_subw_indices, entity_subw_mask,
              triplet_entity_nums, pair_mask, W_fc):
    """Build per-core input maps (numpy only, cheap)."""
    global _LAST_C, _LAST_K, _LAYOUT, _SPILL
    import ml_dtypes
    bf16 = ml_dtypes.bfloat16
    hs = np.asarray(hidden_states, dtype=np.float32)
    idx = np.asarray(entity_subw_indices)
    msk = np.asarray(entity_subw_mask).astype(np.float32)
    trip = np.asarray(triplet_entity_nums)
    pm = np.asarray(pair_mask)
    # permute W rows to slot order, then shuffle to the SBUF chunk layout
    w = (np.asarray(W_fc, dtype=np.float32)[PERMW].reshape(FC, 128, NT)
         .transpose(1, 0, 2).reshape(128, FC * NT).astype(bf16))

    cnt = np.maximum(msk.sum(axis=2), 1.0)          # (B, E)
    wgt = msk / cnt[:, :, None]                     # (B, E, S)

    # distinct hidden rows per example; columns permuted to i-major
    used = [np.unique(idx[b][msk[b] > 0]) for b in range(B)]
    K = max(1, int(np.ceil(max(len(u) for u in used) / 128)))
    KR = K * 128
    hidg = np.zeros((B, KR, H), bf16)
    at2 = np.zeros((B, KR, E), np.float32)
    for b in range(B):
        u = used[b]
        hidg[b, : len(u)] = hs[b][u][:, PERMH].astype(bf16)
        pos = np.full(L, -1, np.int64)
        pos[u] = np.arange(len(u))
        e_i, s_i = np.nonzero(msk[b] > 0)
        np.add.at(at2[b], (pos[idx[b, e_i, s_i]], e_i), wgt[b, e_i, s_i])
    at2 = at2.astype(bf16)

    nv = pm.sum(axis=1).astype(np.int64)
    pairs = _pair_examples(nv)
    maxsum = max(int(nv[a] + nv[b]) for a, b in pairs)
    C = min(128, int(np.ceil(max(maxsum, 16) / 16) * 16))
    _LAST_C, _LAST_K = C, K

    KK = 2 * K
    gg = np.zeros((len(pairs), 128, 3 * C), bf16)
    gs = gg[:, :, :C]
    go = gg[:, :, C : 2 * C]
    gg[0, :C, 2 * C :] = np.eye(C, dtype=bf16)
    for p in range(1, len(pairs)):
        gg[p, :C, 2 * C :] = np.eye(C, dtype=bf16)
    at_pair = np.zeros((len(pairs), KK * 128, 2 * E), bf16)
    hid_pair = np.zeros((len(pairs), KK * 128, H), bf16)
    colmap = []  # per pair: (bs array, ts array)
    spill = []
    for p, (bx, by) in enumerate(pairs):
        hid_pair[p, :KR] = hidg[bx]
        hid_pair[p, KR:] = hidg[by]
        at_pair[p, :KR, :E] = at2[bx]
        at_pair[p, KR:, E:] = at2[by]
        bs, ts = [], []
        k = 0
        for exl, b in ((0, bx), (1, by)):
            tv = np.nonzero(pm[b])[0]
            keep = min(len(tv), C - k)
            for t in tv[keep:]:
                spill.append((b, int(t)))
            tv = tv[:keep]
            n = len(tv)
            gs[p, exl * E + trip[b, tv, 0], k + np.arange(n)] = 1.0
            go[p, exl * E + trip[b, tv, 1], k + np.arange(n)] = 1.0
            bs.append(np.full(n, b))
            ts.append(tv)
            k += n
        colmap.append((np.concatenate(bs), np.concatenate(ts)))
    _LAYOUT = colmap
    _SPILL = [(b, t,
               _host_logits_row(hs, idx, wgt, trip, b, t, W_fc))
              for b, t in spill]

    in_maps = []
    for c in range(NCORES):
        p0 = c * NPAIR
        in_maps.append({
            "hid": np.ascontiguousarray(
                hid_pair[p0 : p0 + NPAIR].reshape(NPAIR * KK * 128, H)),
            "at": np.ascontiguousarray(
                at_pair[p0 : p0 + NPAIR].reshape(NPAIR * KK * 128, 2 * E)),
            "gg": np.ascontiguousarray(gg[p0 : p0 + NPAIR]),
            "w": w,
        })
    return in_maps


def _host_logits_row(hs, idx, wgt, trip, b, t, W_fc):
    """Exact logits (without bias) for one (b, t) triplet."""
    e1, e2 = int(trip[b, t, 0]), int(trip[b, t, 1])
    subj = (wgt[b, e1][:, None] * hs[b, idx[b, e1]]).sum(0)
    obj = (wgt[b, e2][:, None] * hs[b, idx[b, e2]]).sum(0)
    bl = (subj.reshape(96, 8, 1) * obj.reshape(96, 1, 8)).reshape(-1)
    return bl @ np.asarray(W_fc, np.float32)


def assemble(results, b_fc):
    """results[c]["out"] is (2, NT, NPAIR, C) -> (B, T, NT) + bias."""
    bfc = np.asarray(b_fc, np.float32)
    logits = np.broadcast_to(bfc, (B, T, NT)).copy()
    for c in range(NCORES):
        o = np.asarray(results[c]["out"], np.float32)
        ologit = o[0] + o[1]                        # (NT, NPAIR, C)
        for P in range(NPAIR):
            bs, ts = _LAYOUT[c * NPAIR + P]
            n = len(bs)
            logits[bs, ts, :] = ologit[:, P, :n].T + bfc
    for b, t, row in _SPILL:
        logits[b, t, :] = row + bfc
    return logits


_NC_CACHE = {}


def kernel(hidden_states, entity_subw_indices, entity_subw_mask,
           triplet_entity_nums, pair_mask, W_fc, b_fc):
    in_maps = host_prep(hidden_states, entity_subw_indices, entity_subw_mask,
                        triplet_entity_nums, pair_mask, W_fc)
    key = (_LAST_C, _LAST_K)
    if key not in _NC_CACHE:
        _NC_CACHE[key] = build_program()
    nc = _NC_CACHE[key]
    res = run_bass_kernel_spmd(nc, in_maps, core_ids=list(range(NCORES)))
    return assemble(res.results, b_fc)


# revision 42
# speedup vs baseline: 4.2407x; 1.0051x over previous
"""Trainium2 Bass kernel for BilinearClassification (segment_reduce).

Math (per example b):
  ent[e,:]  = masked-mean over subword span of hidden[idx[e,s],:]      (E=64, H=768)
  subj[t,:] = ent[trip[t,0],:] * pm[t];  obj[t,:] = ent[trip[t,1],:] * pm[t]
  bl[t, (g,i,j)] = subj[t, g*8+i] * obj[t, g*8+j]                      (f = 6144)
  logits[t,n] = bl[t,:] @ W[:,n] + b[n]                                (NT=42)

Device strategy (8 cores, 4 examples = 2 example-pairs each, no collectives):
  - Only VALID triplets (pair_mask) are computed, packed into C<=128 columns
    per example-pair; the few overflow triplets are computed on the host.
  - Host gathers only the DISTINCT hidden rows each example references and
    PERMUTES hidden columns to i-major order: col' (i,g) = col (g*8+i).
    In i-major order the bilinear outer product bl[t, slot(i,j,g)] =
    s'[t, i*96+g] * o'[t, j*96+g] has unit-stride innermost dim (g) on
    BOTH operands and the destination -> DVE runs it in 2x perf mode with
    ONE instruction per (pair, i)-slice.  W rows are permuted to match.
  - stage 1: ent'[(ex,e), h'] = AT.T @ hidg per pair (PE).
  - gathers: s' = Gs.T @ ent', o' = Go.T @ ent'  (PE, one-hot matrices).
  - muls: bl_i[t, (j, g)] on DVE (some slices on GPSIMD).
  - bridge to [f, t]: per (pair, i) 6 PE transposes + evac, or one grouped
    XBAR DMA transpose ([C, 768] -> [128, 6, C]).
  - FC: logits[n, t] += W_c.T @ blT_c, 48 chunks of N=2C (both pairs in one
    moving operand), col-tiled 2x on the PE array (even chunks -> partitions
    0:42, odd chunks -> 64:106); halves summed on host.
  - host scatters packed columns back to (b, t) and adds b_fc.
"""
import sys

sys.path.insert(0, "/opt/trn_rl_repo")

import numpy as np

import concourse.bass as bass
import concourse.bacc as bacc
import concourse.tile as tile
from concourse import mybir
from concourse.bass_utils import run_bass_kernel_spmd

F32 = mybir.dt.float32
BF16 = mybir.dt.bfloat16
FP8 = mybir.dt.float8e4
U8 = mybir.dt.uint8

B, L, H = 32, 512, 768
E, S, T = 64, 8, 128
NT = 42
NCORES = 8
EXPC = B // NCORES          # 4 examples per core
NPAIR = EXPC // 2           # 2 example-pairs per core
FC = (H * 8) // 128         # 48 f-chunks
HH = H // 2
NI = 8                      # i-slices per pair (each 6 f-chunks)
NCH = 6                     # f-chunks per i-slice

# data-dependent compile parameters (set by host_prep; defaults match the
# bundled fixed-seed inputs)
_LAST_C = 128               # packed triplet columns per pair (<= 128)
_LAST_K = 2                 # 128-row contraction chunks per example

# (pair, i) units routed to the XBAR DMA transpose instead of PE+evac
DMA_TR = {(P, i) for P in range(2) for i in (0, 2, 4, 6)}
# (pair, i) mul units routed to GPSIMD instead of DVE (early: GPS is slow)
GPS_MULS = {(0, 1), (1, 1), (0, 3), (1, 3)}
# (pair, i) units whose transpose-evac runs on DVE (rest: ACT)
DVE_EVAC = {(0, 1), (0, 5), (1, 3), (1, 7)}
GPS_BUFS = 4
TRPS_BUFS = 2
# FC accumulation visits i-slices in this order (free: flags follow order)
FC_ORDER = [1, 3, 5, 0, 7, 2, 4, 6]
# emission order of the 16 (pair, i) units
UNIT_ORDER = [(0, 0), (0, 1), (0, 2), (1, 0), (0, 3), (1, 1), (0, 4), (1, 2),
              (0, 5), (1, 3), (0, 6), (1, 4), (0, 7), (1, 5), (1, 6), (1, 7)]


def _pack_offsets(C, K):
    """Byte offsets of the packed per-partition consts tensor."""
    KK = 2 * K
    o_at = 0
    o_gg = o_at + NPAIR * KK * 2 * E          # fp8 counts
    o_id = o_gg + NPAIR * 2 * C               # fp8 one-hots
    o_cnt = o_id + 2 * C                      # bf16 identity
    o_w = o_cnt + 4 * NPAIR                   # f32 1/cnt
    o_end = o_w + 2 * FC * NT                 # bf16 W
    return o_at, o_gg, o_id, o_cnt, o_w, o_end

# hidden-column permutation to i-major: hid'[:, i*96+g] = hid[:, g*8+i]
PERMH = np.arange(H).reshape(96, 8).T.reshape(-1)
# W-row permutation to slot order: slot(i,j,g) = i*768 + j*96 + g,
# original f = g*64 + i*8 + j
PERMW = np.arange(H * 8).reshape(96, 8, 8).transpose(1, 2, 0).reshape(-1)


def build_program(reps=1, C=None, K=None):
    C = _LAST_C if C is None else C
    K = _LAST_K if K is None else K
    nc = bacc.Bacc("TRN2", target_bir_lowering=False, debug=False)

    KK = 2 * K              # contraction chunks per pair
    hid_d = nc.dram_tensor("hid", (NPAIR * KK * 128, H), BF16, kind="ExternalInput")
    # at holds subword COUNTS (small ints, fp8-exact); 1/span_len is applied
    # as a per-partition scale during the stage-1 evac
    at_d = nc.dram_tensor("at", (NPAIR * KK * 128, 2 * E), FP8, kind="ExternalInput")
    gg_d = nc.dram_tensor("gg", (NPAIR, 128, 2 * C), FP8, kind="ExternalInput")
    # identity for PE transposes + per-(pair, entity) 1/cnt scales
    idc_d = nc.dram_tensor("idc", (128, C), BF16, kind="ExternalInput")
    cnt_d = nc.dram_tensor("cnt", (128, NPAIR), F32, kind="ExternalInput")
    # W pre-permuted (PERMW) and pre-shuffled to the SBUF chunk layout
    w_d = nc.dram_tensor("w", (128, FC * NT), BF16, kind="ExternalInput")
    # halves: [0] = even-chunk half, [1] = odd-chunk half (host sums)
    out_d = nc.dram_tensor("out", (2, NT, NPAIR, C), F32, kind="ExternalOutput")

    with tile.TileContext(nc) as tc:
        with (
            tc.tile_pool(name="consts", bufs=2) as consts,
            tc.tile_pool(name="hidp", bufs=4) as hidp,
            tc.tile_pool(name="entp", bufs=4) as entp,
            tc.tile_pool(name="sop", bufs=8) as sop,
            tc.tile_pool(name="gps", bufs=GPS_BUFS, space="PSUM") as gps,
            tc.tile_pool(name="trps", bufs=TRPS_BUFS, space="PSUM") as trps,
            tc.tile_pool(name="lgps", bufs=1, space="PSUM") as lgps,
            tc.tile_pool(name="blp", bufs=8) as blp,
            tc.tile_pool(name="blsb", bufs=2) as blsb,
            tc.tile_pool(name="outp", bufs=2) as outp,
        ):
          hid_r = hid_d[:].rearrange("(pk p) h -> p pk h", p=128)
          at_r = at_d[:].rearrange("(pk p) e -> p pk e", p=128)
          gg_r = gg_d[:].rearrange("q p e -> p q e")

          def emit_dmas():
            att = consts.tile([128, NPAIR, KK, 2 * E], FP8, tag="at")
            nc.sync.dma_start(
                att[:], at_r.rearrange("p (q k) e -> p q k e", q=NPAIR))
            gg = consts.tile([128, NPAIR, 2 * C], FP8, tag="gg")
            nc.sync.dma_start(gg[:], gg_r)
            idc = consts.tile([128, C], BF16, tag="idc")
            nc.sync.dma_start(idc[:], idc_d[:])
            cnt = consts.tile([128, NPAIR], F32, tag="cnt")
            nc.sync.dma_start(cnt[:], cnt_d[:])
            w_all = consts.tile([128, FC, NT], BF16, tag="w")
            nc.sync.dma_start(w_all[:], w_d[:].rearrange("p (c n) -> p c n", n=NT))
            hid_t = []
            for P in range(NPAIR):
                ht = hidp.tile([128, KK, H], BF16)
                nc.sync.dma_start(ht[:], hid_r[:, P * KK : (P + 1) * KK, :])
                hid_t.append(ht)
            return att, gg, idc, cnt, w_all, hid_t

          def emit_prologue(dmas):
            att, gg, idc, cnt, w_all, hid_t = dmas
            gs_t = [gg[:][:, P, :C] for P in range(NPAIR)]
            go_t = [gg[:][:, P, C : 2 * C] for P in range(NPAIR)]
            cnt_v = cnt[:]
            ss_l, oo_l = [], []
            for P in range(NPAIR):
                # stage 1: ent'[(ex,e), h']
                esb = entp.tile([128, H], BF16, tag=f"ent{P}")
                for nh in range(2):
                    ps = gps.tile([128, 512], F32, tag="g")
                    ent_ps = ps[:][:, :HH]
                    for ck in range(KK):
                        nc.tensor.matmul(
                            ent_ps,
                            att[:][:, P, ck, :],
                            hid_t[P][:, ck, nh * HH : (nh + 1) * HH],
                            start=(ck == 0),
                            stop=(ck == KK - 1),
                        )
                    nc.scalar.activation(
                        esb[:, nh * HH : (nh + 1) * HH], ent_ps,
                        mybir.ActivationFunctionType.Copy,
                        scale=cnt_v[:, P].unsqueeze(1))
                # gathers: o' first (muls need full o'), then s'
                oo = sop.tile([128, H], BF16, tag=f"oT{P}")
                ss = sop.tile([128, H], BF16, tag=f"sT{P}")
                oo_l.append(oo)
                ss_l.append(ss)
                for gmat, dst in ((go_t[P], oo), (gs_t[P], ss)):
                    for nh in range(2):
                        ps = gps.tile([128, 512], F32, tag="g")
                        t_ps = ps[:][:C, :HH]
                        nc.tensor.matmul(
                            t_ps, gmat,
                            esb[:, nh * HH : (nh + 1) * HH],
                            start=True, stop=True)
                        if nh == 0:
                            nc.scalar.copy(dst[:C, :HH], t_ps)
                        elif dst is oo:
                            nc.scalar.copy(dst[:C, HH:], t_ps)
                        else:
                            nc.vector.tensor_copy(dst[:C, HH:], t_ps)
            return ss_l, oo_l

          cur = emit_dmas()
          pro = emit_prologue(cur)
          emitted = [cur]
          for _rep in range(reps):
            att, gg, idc, cnt, w_all, hid_t = cur
            ss_l, oo_l = pro
            if _rep + 1 < reps:
                cur = emit_dmas()
            id_t = idc[:][:C, :]

            # ---- body: muls -> transposes -> FC for this rep
            lgA = lgps.tile([128, 512], F32, tag="lgA")
            lgB = lgps.tile([128, 512], F32, tag="lgB")
            blT = [blsb.tile([128, NCH, NPAIR, C], BF16, name=f"blT{_i}")
                   for _i in range(NI)]
            # muls + transposes, custom unit order (P0 leads so P1's
            # prologue latency hides; FC(i) needs both pairs of blT[i])
            for (P, i) in UNIT_ORDER:
                    ss, oo = ss_l[P], oo_l[P]
                    # bl_i[t, (j, g)] = s'[t, i*96+g] * o'[t, j*96+g]
                    bl = blp.tile([128, 8, 96], BF16)
                    s_op = (ss[:C, i * 96 : (i + 1) * 96]
                            .unsqueeze(1).broadcast_to((C, 8, 96)))
                    o_op = oo[:C].rearrange("p (j g) -> p j g", g=96)
                    dst = bl[:C]
                    if (P, i) in GPS_MULS:
                        nc.gpsimd.tensor_mul(dst, s_op, o_op)
                    else:
                        nc.vector.tensor_mul(dst, s_op, o_op)
                    bl2 = bl[:C].rearrange("p a b -> p (a b)")
                    if (P, i) in DMA_TR:
                        nc.sync.dma_start(
                            blT[i][:][:, :, P, :], bl2, transpose=True)
                    else:
                        trp = trps.tile([128, NCH, C], BF16, tag="tr")
                        for m in range(NCH):
                            nc.tensor.transpose(
                                trp[:][:, m, :C],
                                bl2[:, m * 128 : (m + 1) * 128], id_t)
                        if (P, i) in DVE_EVAC:
                            nc.vector.tensor_copy(
                                blT[i][:][:, :, P, :], trp[:])
                        else:
                            nc.scalar.copy(blT[i][:][:, :, P, :], trp[:])
            # FC: 6 chunks per i-slice, both pairs per matmul
            for ii, i in enumerate(FC_ORDER):
                for m in range(NCH):
                    q = i * NCH + m
                    half = (ii * NCH + m) % 2
                    kk = (ii * NCH + m) // 2
                    out_ap = (lgA[:][0:42, : NPAIR * C] if half == 0
                              else lgB[:][64:106, : NPAIR * C])
                    nc.tensor.matmul(
                        out_ap,
                        w_all[:][:, q, :],
                        blT[i][:][:, m, :, :],
                        start=(kk == 0),
                        stop=(kk == 23),
                    )
            if _rep + 1 < reps:
                pro = emit_prologue(cur)

            out_sb = outp.tile([128, NPAIR, C], F32)
            nc.scalar.copy(
                out_sb[:][0:42].rearrange("p q c -> p (q c)"),
                lgA[:][0:42, : NPAIR * C])
            nc.vector.tensor_copy(
                out_sb[:][64:106].rearrange("p q c -> p (q c)"),
                lgB[:][64:106, : NPAIR * C])
            nc.sync.dma_start(out_d[0], out_sb[:][0:NT])
            nc.sync.dma_start(out_d[1], out_sb[:][64 : 64 + NT])

    nc.compile()
    return nc


def _pair_examples(nv):
    """Pair up examples to minimize the max pair sum (greedy fold + local
    search over pairwise re-pairings)."""
    nv = np.asarray(nv)
    order = list(np.argsort(-nv))
    n = len(order) // 2
    pairs = [[order[i], order[2 * n - 1 - i]] for i in range(n)]

    def ps(p):
        return int(nv[p[0]] + nv[p[1]])

    changed = True
    it = 0
    while changed and it < 1000:
        changed = False
        it += 1
        for i in range(n):
            for j in range(i + 1, n):
                a, b = pairs[i], pairs[j]
                cur = max(ps(a), ps(b))
                for (x, y) in (((a[0], b[0]), (a[1], b[1])),
                               ((a[0], b[1]), (a[1], b[0]))):
                    m = max(int(nv[x[0]] + nv[x[1]]), int(nv[y[0]] + nv[y[1]]))
                    if m < cur:
                        pairs[i], pairs[j] = list(x), list(y)
                        a, b = pairs[i], pairs[j]
                        cur = m
                        changed = True
    return pairs


# layout metadata shared between host_prep / assemble / kernel
_LAYOUT = None      # per pair: (bs, ts) arrays for packed columns
_SPILL = None       # list of (b, t, logits_row) computed on host


def host_prep(hidden_states, entity_subw_indices, entity_subw_mask,
              triplet_entity_nums, pair_mask, W_fc):
    """Build per-core input maps (numpy only, cheap)."""
    global _LAST_C, _LAST_K, _LAYOUT, _SPILL
    import ml_dtypes
    bf16 = ml_dtypes.bfloat16
    fp8 = ml_dtypes.float8_e4m3
    hs = np.asarray(hidden_states, dtype=np.float32)
    idx = np.asarray(entity_subw_indices)
    msk = np.asarray(entity_subw_mask).astype(np.float32)
    trip = np.asarray(triplet_entity_nums)
    pm = np.asarray(pair_mask)
    # permute W rows to slot order, then shuffle to the SBUF chunk layout
    w = (np.asarray(W_fc, dtype=np.float32)[PERMW].reshape(FC, 128, NT)
         .transpose(1, 0, 2).reshape(128, FC * NT).astype(bf16))

    cnt = np.maximum(msk.sum(axis=2), 1.0)          # (B, E)
    wgt = msk / cnt[:, :, None]                     # (B, E, S)

    # distinct hidden rows per example; columns permuted to i-major.
    # at2 holds subword COUNTS (exact small ints); 1/cnt applied on-device.
    used = [np.unique(idx[b][msk[b] > 0]) for b in range(B)]
    K = max(1, int(np.ceil(max(len(u) for u in used) / 128)))
    KR = K * 128
    hidg = np.zeros((B, KR, H), bf16)
    at2 = np.zeros((B, KR, E), np.float32)
    for b in range(B):
        u = used[b]
        hidg[b, : len(u)] = hs[b][u][:, PERMH].astype(bf16)
        pos = np.full(L, -1, np.int64)
        pos[u] = np.arange(len(u))
        e_i, s_i = np.nonzero(msk[b] > 0)
        np.add.at(at2[b], (pos[idx[b, e_i, s_i]], e_i), 1.0)
    at2 = at2.astype(fp8)

    nv = pm.sum(axis=1).astype(np.int64)
    pairs = _pair_examples(nv)
    maxsum = max(int(nv[a] + nv[b]) for a, b in pairs)
    C = min(128, int(np.ceil(max(maxsum, 16) / 16) * 16))
    _LAST_C, _LAST_K = C, K

    KK = 2 * K
    gg = np.zeros((len(pairs), 128, 2 * C), fp8)
    gs = gg[:, :, :C]
    go = gg[:, :, C : 2 * C]
    idc = np.eye(128, dtype=bf16)[:, :C]
    cntmap = np.ones((len(pairs), 128), np.float32)
    at_pair = np.zeros((len(pairs), KK, 128, 2 * E), fp8)
    at_flat = at_pair.reshape(len(pairs), KK * 128, 2 * E)
    hid_pair = np.zeros((len(pairs), KK * 128, H), bf16)
    colmap = []  # per pair: (bs array, ts array)
    spill = []
    for p, (bx, by) in enumerate(pairs):
        hid_pair[p, :KR] = hidg[bx]
        hid_pair[p, KR:] = hidg[by]
        at_flat[p, :KR, :E] = at2[bx]
        at_flat[p, KR:, E:] = at2[by]
        cntmap[p, :E] = 1.0 / cnt[bx]
        cntmap[p, E:2 * E] = 1.0 / cnt[by]
        bs, ts = [], []
        k = 0
        for exl, b in ((0, bx), (1, by)):
            tv = np.nonzero(pm[b])[0]
            keep = min(len(tv), C - k)
            for t in tv[keep:]:
                spill.append((b, int(t)))
            tv = tv[:keep]
            n = len(tv)
            gs[p, exl * E + trip[b, tv, 0], k + np.arange(n)] = 1.0
            go[p, exl * E + trip[b, tv, 1], k + np.arange(n)] = 1.0
            bs.append(np.full(n, b))
            ts.append(tv)
            k += n
        colmap.append((np.concatenate(bs), np.concatenate(ts)))
    _LAYOUT = colmap
    _SPILL = [(b, t,
               _host_logits_row(hs, idx, wgt, trip, b, t, W_fc))
              for b, t in spill]

    in_maps = []
    for c in range(NCORES):
        p0 = c * NPAIR
        in_maps.append({
            "hid": np.ascontiguousarray(
                hid_pair[p0 : p0 + NPAIR].reshape(NPAIR * KK * 128, H)),
            "at": np.ascontiguousarray(
                at_pair[p0 : p0 + NPAIR].reshape(NPAIR * KK * 128, 2 * E)),
            "gg": np.ascontiguousarray(gg[p0 : p0 + NPAIR]),
            "idc": idc,
            "cnt": np.ascontiguousarray(cntmap[p0 : p0 + NPAIR].T),
            "w": w,
        })
    return in_maps


def _host_logits_row(hs, idx, wgt, trip, b, t, W_fc):
    """Exact logits (without bias) for one (b, t) triplet."""
    e1, e2 = int(trip[b, t, 0]), int(trip[b, t, 1])
    subj = (wgt[b, e1][:, None] * hs[b, idx[b, e1]]).sum(0)
    obj = (wgt[b, e2][:, None] * hs[b, idx[b, e2]]).sum(0)
    bl = (subj.reshape(96, 8, 1) * obj.reshape(96, 1, 8)).reshape(-1)
    return bl @ np.asarray(W_fc, np.float32)


def assemble(results, b_fc):
    """results[c]["out"] is (2, NT, NPAIR, C) -> (B, T, NT) + bias."""
    bfc = np.asarray(b_fc, np.float32)
    logits = np.broadcast_to(bfc, (B, T, NT)).copy()
    for c in range(NCORES):
        o = np.asarray(results[c]["out"], np.float32)
        ologit = o[0] + o[1]                        # (NT, NPAIR, C)
        for P in range(NPAIR):
            bs, ts = _LAYOUT[c * NPAIR + P]
            n = len(bs)
            logits[bs, ts, :] = ologit[:, P, :n].T + bfc
    for b, t, row in _SPILL:
        logits[b, t, :] = row + bfc
    return logits


_NC_CACHE = {}


def kernel(hidden_states, entity_subw_indices, entity_subw_mask,
           triplet_entity_nums, pair_mask, W_fc, b_fc):
    in_maps = host_prep(hidden_states, entity_subw_indices, entity_subw_mask,
                        triplet_entity_nums, pair_mask, W_fc)
    key = (_LAST_C, _LAST_K)
    if key not in _NC_CACHE:
        _NC_CACHE[key] = build_program()
    nc = _NC_CACHE[key]
    res = run_bass_kernel_spmd(nc, in_maps, core_ids=list(range(NCORES)))
    return assemble(res.results, b_fc)
